# revision 1
# baseline (speedup 1.0000x reference)
"""AnomalyTransformer forward pass on 8 Trainium2 NeuronCores.

Data-parallel over batch: each core processes 32 of the 256 batch elements
through the full 3-layer transformer.

Precision strategy: the residual stream h and all projection weights on
the q/k path run in float32r (TF32-like TensorEngine mode, full throughput
at moving-dim >= 256); q/k chunk tiles, the value path and attention
output run in bf16. Softmax logits reach +-38 in layer 3, so an all-bf16
kernel amplifies rounding to ~1.5e-2 relative error; this mix lands at
~6.1e-3 (measured) against the f32 reference, with the Tile cost-model
timeline at ~522 us per core.

Layout strategy: the residual stream h is feature-major ([D, tokens], D
split over 4 partition-tiles of 128). Attention uses the scoresT
orientation (scoresT = khT.T @ qhT -> [l_k, l_q]) so softmax normalization
folds into the attention-value matmul via an appended ones-column on V
(column 64 of each head's 65-wide slot accumulates sum(exp)); the
per-token reciprocal is then a per-partition scalar multiply. One
[100, 512] PE-transpose per batch element brings the attention output back
to feature-major for the Wo projection. Residual adds are folded into the
Wo/W2 PSUM accumulation as identity matmuls.

The sigma/prior branch of the reference is dead code (never feeds the
output) and is skipped. Biases in the reference are all zeros and are
skipped.
"""

import sys
import os
for _p in ("/opt/trn_rl_repo", "/root/.axon_site/_ro/trn_rl_repo"):
    if os.path.isdir(_p) and _p not in sys.path:
        sys.path.insert(0, _p)

import math
import numpy as np
import ml_dtypes

import concourse.bass as bass
import concourse.tile as tile
from concourse import mybir
from concourse.bass_utils import run_bass_kernel_spmd
from contextlib import ExitStack

BF16 = mybir.dt.bfloat16
F32 = mybir.dt.float32
F32R = mybir.dt.bfloat16 if os.environ.get("ANOM_RDT", "f32r") == "bf16" else mybir.dt.float32r
AF = mybir.ActivationFunctionType
OP = mybir.AluOpType

# model dims
B, L, C, D, H, NL, DFF = 256, 100, 55, 512, 8, 3, 64
DK = D // H                      # 64
NCORES = 8
BL = B // NCORES                 # 32 batches per core
TOK = BL * L                     # 3200 tokens per core
TCH = 400                        # token chunk (4 batches)
NT = TOK // TCH                  # 8 chunks
CB = TCH // L                    # 4 batches per chunk
KT = D // 128                    # 4 contraction tiles
C3 = 3 * C                       # 165 unfolded conv rows


_NOSTRUCT = ("InstDrain", "InstNoOp", "InstEventSemaphore", "InstHalt")


def _legalize_waits(nc, maxw=1):
    """This container's walrus caps sync-waits at 1 per instruction; move
    extra waits onto preceding same-engine NOPs (one wait each)."""
    cnt = [0]
    for f in nc.m.functions:
        for blk in f.blocks:
            newlist = []
            changed = False
            for ins in blk.instructions:
                si = getattr(ins, "sync_info", None)
                lim = maxw
                if si is not None and si.on_wait and len(si.on_wait) > lim:
                    waits = list(si.on_wait)
                    extra, keep = waits[:-lim], waits[-lim:]
                    for i in range(0, len(extra), 1):
                        cnt[0] += 1
                        nop = mybir.InstNoOp(
                            name=f"I-ws-{cnt[0]}", ins=[], outs=[], engine=ins.engine
                        )
                        nop.sync_info = mybir.SyncInfo(
                            on_wait=extra[i:i + 1], on_update=[]
                        )
                        newlist.append(nop)
                    ins.sync_info = mybir.SyncInfo(
                        on_wait=keep, on_update=list(si.on_update)
                    )
                    changed = True
                newlist.append(ins)
            if changed:
                blk.instructions = newlist
    return nc


def _offset_ap(ap, extra_offset, dims):
    """AP at ap.offset + extra_offset (elements) with free dims `dims`
    ([[step, count], ...]), keeping ap's partition dim."""
    return bass.AP(tensor=ap.tensor, offset=ap.offset + extra_offset,
                   ap=[list(ap.ap[0])] + [list(d) for d in dims])


def build_nc():
    RES_DVE = os.environ.get("ANOM_RESDVE", "1") == "1"
    QK_BF = os.environ.get("ANOM_QKBF", "1") == "1"
    QKDT = BF16 if QK_BF else F32R
    QK_ACT = os.environ.get("ANOM_QKACT", "1") == "1"
    OT_DVE = os.environ.get("ANOM_OTDVE", "1") == "1"
    V_ACT = os.environ.get("ANOM_VACT", "0") == "1"
    nc = bass.Bass()

    # ---- DRAM parameters (host-prepped) ----
    xcat0 = nc.declare_dram_parameter("xcat0", [128, TOK], F32R, isOutput=False)
    xcat1 = nc.declare_dram_parameter("xcat1", [C3 - 128, TOK], F32R, isOutput=False)
    wemb0 = nc.declare_dram_parameter("wemb0", [128, D], F32R, isOutput=False)
    wemb1 = nc.declare_dram_parameter("wemb1", [C3 - 128, D], F32R, isOutput=False)
    pe_d = nc.declare_dram_parameter("pe", [128, KT, L], F32, isOutput=False)
    wq_d = nc.declare_dram_parameter("wq", [128, NL, KT, D], F32R, isOutput=False)
    wk_d = nc.declare_dram_parameter("wk", [128, NL, KT, D], F32R, isOutput=False)
    wv_d = nc.declare_dram_parameter("wv", [128, NL, KT, D], F32R, isOutput=False)
    wo_d = nc.declare_dram_parameter("wo", [128, NL, KT, D], BF16, isOutput=False)
    w1_d = nc.declare_dram_parameter("w1", [128, NL, KT, DFF], F32R, isOutput=False)
    w2_d = nc.declare_dram_parameter("w2", [DFF, NL, KT, 128], F32R, isOutput=False)
    wout_d = nc.declare_dram_parameter("wout", [128, KT, C], F32R, isOutput=False)
    identb_d = nc.declare_dram_parameter("identb", [128, 128], BF16, isOutput=False)
    identr_d = nc.declare_dram_parameter("identr", [128, 128], F32R, isOutput=False)
    out_d = nc.declare_dram_parameter("out", [C, TOK], F32, isOutput=True)

    with tile.TileContext(nc) as tc, ExitStack() as stk:
        tc.race_detector_enabled = False
        singles = stk.enter_context(tc.tile_pool(name="singles", bufs=1))
        wp = stk.enter_context(tc.tile_pool(name="wp", bufs=int(os.environ.get("ANOM_WPB", "2"))))
        xp = stk.enter_context(tc.tile_pool(name="xp", bufs=int(os.environ.get("ANOM_XPB", "3"))))
        qp = stk.enter_context(tc.tile_pool(name="qp", bufs=int(os.environ.get("ANOM_QB", "2"))))
        kp = stk.enter_context(tc.tile_pool(name="kp", bufs=int(os.environ.get("ANOM_QB", "2"))))
        vp = stk.enter_context(tc.tile_pool(name="vp", bufs=int(os.environ.get("ANOM_VB", "2"))))
        expp = stk.enter_context(tc.tile_pool(name="expp", bufs=int(os.environ.get("ANOM_EXB", "3"))))
        op_ = stk.enter_context(tc.tile_pool(name="op", bufs=int(os.environ.get("ANOM_OB", "3"))))
        rp = stk.enter_context(tc.tile_pool(name="rp", bufs=3))
        otp = stk.enter_context(tc.tile_pool(name="otp", bufs=int(os.environ.get("ANOM_OTB", "2"))))
        yp = stk.enter_context(tc.tile_pool(name="yp", bufs=2))
        outp = stk.enter_context(tc.tile_pool(name="outp", bufs=2))
        # psum pools (8 banks total)
        pp = stk.enter_context(tc.tile_pool(name="pp", bufs=int(os.environ.get("ANOM_PPB", "3")), space="PSUM"))
        scp = stk.enter_context(tc.tile_pool(name="scp", bufs=int(os.environ.get("ANOM_SCB", "3")), space="PSUM"))
        oup = stk.enter_context(tc.tile_pool(name="oup", bufs=int(os.environ.get("ANOM_OUB", "1")), space="PSUM"))
        tpp = stk.enter_context(tc.tile_pool(name="tpp", bufs=int(os.environ.get("ANOM_TPB", "1")), space="PSUM"))

        # ---- persistent SBUF ----
        wemb0_sb = singles.tile([128, D], F32R)
        wemb1_sb = singles.tile([C3 - 128, D], F32R)
        pe_sb = singles.tile([128, KT, L], F32)
        wout_sb = singles.tile([128, KT, C], F32R)
        ident_b = singles.tile([128, 128], BF16)
        ident_r = singles.tile([128, 128], F32R)
        h_sb = [singles.tile([128, TOK], F32R, name=f"h{k}") for k in range(KT)]

        for dst, src in ((wemb0_sb, wemb0), (wemb1_sb, wemb1), (pe_sb, pe_d),
                         (wout_sb, wout_d), (ident_b, identb_d),
                         (ident_r, identr_d)):
            nc.sync.dma_start(out=dst[:], in_=src[:])

        # ---- token embedding: circular conv as matmul, + positional emb ----
        for t in range(NT):
            tsl = slice(t * TCH, (t + 1) * TCH)
            xc0 = xp.tile([128, TCH], F32R, tag="xc0")
            xc1 = xp.tile([C3 - 128, TCH], F32R, tag="xc1")
            nc.sync.dma_start(out=xc0[:], in_=xcat0[:, tsl])
            nc.sync.dma_start(out=xc1[:], in_=xcat1[:, tsl])
            for m in range(KT):
                ps = pp.tile([128, 512], F32, tag="pp")
                nc.tensor.matmul(ps[:, :TCH], wemb0_sb[:, m * 128:(m + 1) * 128],
                                 xc0[:], start=True, stop=False)
                nc.tensor.matmul(ps[:, :TCH], wemb1_sb[:, m * 128:(m + 1) * 128],
                                 xc1[:], start=False, stop=True)
                pe_b = _offset_ap(pe_sb[:, m, :], 0, [[0, CB], [1, L]])
                nc.vector.tensor_tensor(
                    h_sb[m][:, tsl].rearrange("p (b x) -> p b x", x=L),
                    ps[:, :TCH].rearrange("p (b x) -> p b x", x=L),
                    pe_b, op=OP.add)

        # ---- transformer layers ----
        n_layer_passes = int(os.environ.get("ANOM_LAYERS", str(NL)))
        for lp_i in range(n_layer_passes):
            l = lp_i % NL
            wq_l = wp.tile([128, KT, D], F32R, tag="wq")
            wk_l = wp.tile([128, KT, D], F32R, tag="wk")
            wv_l = wp.tile([128, KT, D], F32R, tag="wv")
            wo_l = wp.tile([128, KT, D], BF16, tag="wo")
            w1_l = wp.tile([128, KT, DFF], F32R, tag="w1")
            w2_l = wp.tile([DFF, KT, 128], F32R, tag="w2")
            for dst, src in ((wq_l, wq_d), (wk_l, wk_d), (wv_l, wv_d),
                             (wo_l, wo_d), (w1_l, w1_d), (w2_l, w2_d)):
                nc.sync.dma_start(out=dst[:], in_=src[:, l])

            for g in range(NT):
                gsl = slice(g * TCH, (g + 1) * TCH)
                # Q/K projections for this chunk, feature-major [D, TCH]
                qc = [qp.tile([128, TCH], QKDT, name=f"qc{m}", tag=f"qc{m}")
                      for m in range(KT)]
                kc = [kp.tile([128, TCH], QKDT, name=f"kc{m}", tag=f"kc{m}")
                      for m in range(KT)]
                for m in range(KT):
                    msl = slice(m * 128, (m + 1) * 128)
                    ps = pp.tile([128, 512], F32, tag="pp")
                    for k in range(KT):
                        nc.tensor.matmul(ps[:, :TCH], wq_l[:, k, msl],
                                         h_sb[k][:, gsl],
                                         start=(k == 0), stop=(k == KT - 1))
                    (nc.scalar.copy if QK_ACT else nc.vector.tensor_copy)(qc[m][:], ps[:, :TCH])
                for m in range(KT):
                    msl = slice(m * 128, (m + 1) * 128)
                    ps = pp.tile([128, 512], F32, tag="pp")
                    for k in range(KT):
                        nc.tensor.matmul(ps[:, :TCH], wk_l[:, k, msl],
                                         h_sb[k][:, gsl],
                                         start=(k == 0), stop=(k == KT - 1))
                    (nc.scalar.copy if QK_ACT else nc.vector.tensor_copy)(kc[m][:], ps[:, :TCH])

                # V projection, token-major per batch (65-stride heads,
                # col 65h+64 = 1 for the softmax-sum trick)
                v_t = vp.tile([128, CB, 8 * 65], BF16, tag="v")
                nc.vector.memset(
                    v_t[:L, :, :].rearrange(
                        "p b (h x) -> p b h x", x=65)[:, :, :, 64:65], 1.0)
                if os.environ.get("ANOM_VMERGE", "0") != "1":
                    for bi in range(CB):
                        b = g * CB + bi
                        bsl = slice(b * L, (b + 1) * L)
                        VP_SEP = os.environ.get("ANOM_VPSEP", "1") == "1"
                        ps = (tpp if VP_SEP else pp).tile(
                            [128, 512], F32, tag="vps" if VP_SEP else "pp",
                            name="vps")
                        for k in range(KT):
                            nc.tensor.matmul(ps[:L, :], h_sb[k][:, bsl],
                                             wv_l[:, k, :],
                                             start=(k == 0), stop=(k == KT - 1))
                        if os.environ.get("ANOM_VSPLIT", "0") == "1":
                            nc.vector.tensor_copy(
                                v_t[:L, bi, :260].rearrange(
                                    "p (h x) -> p h x", x=65)[:, :, :64],
                                ps[:L, :256].rearrange("p (h x) -> p h x", x=64))
                            nc.scalar.copy(
                                _offset_ap(v_t[:L, bi, :], 260, [[65, 4], [1, 64]]),
                                ps[:L, 256:].rearrange("p (h x) -> p h x", x=64))
                        else:
                            (nc.scalar.copy if V_ACT else nc.vector.tensor_copy)(
                                v_t[:L, bi, :].rearrange(
                                    "p (h x) -> p h x", x=65)[:, :, :64],
                                ps[:L, :].rearrange("p (h x) -> p h x", x=64))
                ot_all = otp.tile([128, KT, TCH], BF16, tag="ot")
                for bi in range(CB):
                    b = g * CB + bi
                    bsl = slice(b * L, (b + 1) * L)
                    if os.environ.get("ANOM_VMERGE", "0") == "1":
                        VP_SEP = os.environ.get("ANOM_VPSEP", "1") == "1"
                        ps = (tpp if VP_SEP else pp).tile(
                            [128, 512], F32, tag="vps" if VP_SEP else "pp",
                            name="vps")
                        for k in range(KT):
                            nc.tensor.matmul(ps[:L, :], h_sb[k][:, bsl],
                                             wv_l[:, k, :],
                                             start=(k == 0), stop=(k == KT - 1))
                        if os.environ.get("ANOM_VSPLIT", "0") == "1":
                            nc.vector.tensor_copy(
                                v_t[:L, bi, :260].rearrange(
                                    "p (h x) -> p h x", x=65)[:, :, :64],
                                ps[:L, :256].rearrange("p (h x) -> p h x", x=64))
                            nc.scalar.copy(
                                _offset_ap(v_t[:L, bi, :], 260, [[65, 4], [1, 64]]),
                                ps[:L, 256:].rearrange("p (h x) -> p h x", x=64))
                        else:
                            (nc.scalar.copy if V_ACT else nc.vector.tensor_copy)(
                                v_t[:L, bi, :].rearrange(
                                    "p (h x) -> p h x", x=65)[:, :, :64],
                                ps[:L, :].rearrange("p (h x) -> p h x", x=64))
                    csl = slice(bi * L, (bi + 1) * L)
                    # scoresT for 8 heads: even heads -> scA, odd -> scB
                    # (different PE row groups must write different PSUM banks)
                    if os.environ.get("ANOM_SCSHARE", "0") == "1":
                        scA = pp.tile([128, 512], F32, tag="pp", name="scA")
                        scB = pp.tile([128, 512], F32, tag="pp", name="scB")
                    else:
                        if os.environ.get("ANOM_SCSPLIT", "0") == "1":
                            scA = scp.tile([128, 512], F32, tag="scA")
                            scB = scp.tile([128, 512], F32, tag="scB")
                        else:
                            scA = scp.tile([128, 512], F32, tag="sc")
                            scB = scp.tile([128, 512], F32, tag="sc")
                    for hh in range(8):
                        kt_i, base = divmod(hh * DK, 128)
                        sc = scA if hh % 2 == 0 else scB
                        col = (hh // 2) * 128
                        nc.tensor.matmul(sc[:L, col:col + L],
                                         kc[kt_i][base:base + DK, csl],
                                         qc[kt_i][base:base + DK, csl],
                                         start=True, stop=True)
                    exp_t = expp.tile([128, 8 * L], BF16, tag="exp")
                    # exp; head hh lands at exp_t cols hh*L
                    if os.environ.get("ANOM_EXP4", "0") == "1":
                        for half in range(2):
                            nc.scalar.activation(
                                _offset_ap(exp_t[:L, :], half * 4 * L,
                                           [[2 * L, 2], [1, L]]),
                                _offset_ap(scA[:L, :], half * 256,
                                           [[128, 2], [1, L]]),
                                AF.Exp)
                            nc.scalar.activation(
                                _offset_ap(exp_t[:L, :], half * 4 * L + L,
                                           [[2 * L, 2], [1, L]]),
                                _offset_ap(scB[:L, :], half * 256,
                                           [[128, 2], [1, L]]),
                                AF.Exp)
                    else:
                        nc.scalar.activation(
                            exp_t[:L, :].rearrange("p (h x) -> p h x", x=2 * L)[:, :, :L],
                            scA[:L, :].rearrange("p (h x) -> p h x", x=128)[:, :, :L],
                            AF.Exp)
                        nc.scalar.activation(
                            _offset_ap(exp_t[:L, :], L, [[2 * L, 4], [1, L]]),
                            scB[:L, :].rearrange("p (h x) -> p h x", x=128)[:, :, :L],
                            AF.Exp)
                    # oU = expST.T @ [v | 1]  (token-major, col 64 = sum(exp))
                    OU2 = os.environ.get("ANOM_OU2", "0") == "1"
                    if OU2:
                        ou2 = oup.tile([128, 1024], F32, tag="ou2")
                        ouA = ou2[:, :512]
                        ouB = ou2[:, 512:]
                    elif os.environ.get("ANOM_OUSHARE", "1") == "1":
                        ouA = pp.tile([128, 512], F32, tag="pp", name="ouA")
                        ouB = pp.tile([128, 512], F32, tag="pp", name="ouB")
                    else:
                        ouA = oup.tile([128, 512], F32, tag="ou")
                        ouB = oup.tile([128, 512], F32, tag="ou")
                    for hh in range(8):
                        ou = ouA if hh % 2 == 0 else ouB
                        col = (hh // 2) * 128
                        nc.tensor.matmul(ou[:L, col:col + 65],
                                         exp_t[:L, hh * L:(hh + 1) * L],
                                         v_t[:L, bi, hh * 65:(hh + 1) * 65],
                                         start=True, stop=True)
                    r_t = rp.tile([128, 8], F32, tag="r")
                    o_t = op_.tile([128, D], BF16, tag="o")
                    OB_ACT = os.environ.get("ANOM_OBACT", "0") == "1"
                    if OU2:
                        nc.vector.reciprocal(
                            r_t[:L, :8],
                            ou2[:L, :].rearrange(
                                "p (h x) -> p h x", x=128)[:, :, 64:65])
                        nc.vector.tensor_tensor(
                            o_t[:L, :].rearrange("p (h x) -> p h x", x=64),
                            ou2[:L, :].rearrange(
                                "p (h x) -> p h x", x=128)[:, :, :64],
                            r_t[:L, :8].rearrange(
                                "p (h x) -> p h x", x=1).broadcast_to([L, 8, 64]),
                            op=OP.mult)
                    else:
                      for i, ou in enumerate((ouA, ouB)):
                          nc.vector.reciprocal(
                              r_t[:L, i * 4:(i + 1) * 4],
                              ou[:L, :].rearrange(
                                  "p (h x) -> p h x", x=128)[:, :, 64:65])
                          if OB_ACT and i == 1:
                              # odd heads: 4 ScalarE scale-copies run parallel
                              # with the DVE multiply of the even heads
                              for j in range(4):
                                  nc.scalar.activation(
                                      o_t[:L, 256 + j * 64:256 + (j + 1) * 64],
                                      ou[:L, j * 128:j * 128 + 64],
                                      AF.Copy,
                                      scale=r_t[:L, 4 + j:5 + j])
                          else:
                              nc.vector.tensor_tensor(
                                  o_t[:L, i * 256:(i + 1) * 256].rearrange(
                                      "p (h x) -> p h x", x=64),
                                  ou[:L, :].rearrange(
                                      "p (h x) -> p h x", x=128)[:, :, :64],
                                  r_t[:L, i * 4:(i + 1) * 4].rearrange(
                                      "p (h x) -> p h x", x=1).broadcast_to([L, 4, 64]),
                                  op=OP.mult)
                    # transpose o back to feature-major: all 4 m-chunks into
                    # one psum tile, then one strided copy into ot_all
                    if os.environ.get("ANOM_TPSHARE", "1") == "1":
                        tp = pp.tile([128, 1024], BF16, tag="pp", name="tp")
                    else:
                        tp = tpp.tile([128, 1024], BF16, tag="tp")
                    for m in range(KT):
                        nc.tensor.transpose(tp[:, m * L:(m + 1) * L],
                                            o_t[:L, m * 128:(m + 1) * 128],
                                            ident_b[:L, :L])
                    (nc.vector.tensor_copy if OT_DVE else nc.scalar.copy)(
                        _offset_ap(ot_all[:, :, :], bi * L, [[TCH, KT], [1, L]]),
                        tp[:, :KT * L].rearrange("p (m x) -> p m x", x=L))
                # Wo projection + residual (identity matmul accumulates h)
                for m in range(KT):
                    msl = slice(m * 128, (m + 1) * 128)
                    ps = pp.tile([128, 512], F32, tag="pp")
                    RHALF = os.environ.get("ANOM_RHALF", "0") == "1"
                    if RES_DVE and (not RHALF or m < 2) and os.environ.get("ANOM_RSPLIT", "0") != "1":
                        for k in range(KT):
                            nc.tensor.matmul(ps[:, :TCH], wo_l[:, k, msl],
                                             ot_all[:, k, :],
                                             start=(k == 0), stop=(k == KT - 1))
                        nc.vector.tensor_tensor(h_sb[m][:, gsl], ps[:, :TCH],
                                                h_sb[m][:, gsl], op=OP.add)
                    elif RES_DVE and RHALF:
                        for k in range(KT):
                            nc.tensor.matmul(ps[:, :TCH], wo_l[:, k, msl],
                                             ot_all[:, k, :],
                                             start=(k == 0), stop=False)
                        nc.tensor.matmul(ps[:, :TCH], ident_r[:], h_sb[m][:, gsl],
                                         start=False, stop=True)
                        nc.scalar.copy(h_sb[m][:, gsl], ps[:, :TCH])
                    elif RES_DVE:
                        for k in range(KT):
                            nc.tensor.matmul(ps[:, :TCH], wo_l[:, k, msl],
                                             ot_all[:, k, :],
                                             start=(k == 0), stop=False)
                        nc.tensor.matmul(ps[:, :TCH], ident_r[:], h_sb[m][:, gsl],
                                         start=False, stop=True)
                        nc.scalar.copy(h_sb[m][:, gsl], ps[:, :TCH])
                    else:
                        for k in range(KT):
                            nc.tensor.matmul(ps[:, :TCH], wo_l[:, k, msl],
                                             ot_all[:, k, :],
                                             start=(k == 0), stop=False)
                        nc.tensor.matmul(ps[:, :TCH], ident_r[:], h_sb[m][:, gsl],
                                         start=False, stop=True)
                        nc.scalar.copy(h_sb[m][:, gsl], ps[:, :TCH])
            # FFN phase (separate from attention so the ACT engine loads
            # the Gelu table set once per layer instead of per chunk)
            FFNP = os.environ.get("ANOM_FFNP", "1") == "1"
            for g in range(NT):
                gsl = slice(g * TCH, (g + 1) * TCH)
                ps1 = (oup if FFNP else pp).tile([128, 512], F32,
                                                 tag="ffn" if FFNP else "pp",
                                                 name="ps1")
                for k in range(KT):
                    nc.tensor.matmul(ps1[:DFF, :TCH], w1_l[:, k, :],
                                     h_sb[k][:, gsl],
                                     start=(k == 0), stop=(k == KT - 1))
                y_t = yp.tile([DFF, TCH], F32R, tag="y")
                nc.scalar.activation(y_t[:, :], ps1[:DFF, :TCH], AF.Gelu)
                for m in range(KT):
                    ps2 = (oup if FFNP else pp).tile([128, 512], F32,
                                                     tag="ffn" if FFNP else "pp",
                                                     name="ps2")
                    if RES_DVE and (os.environ.get("ANOM_RHALF", "0") != "1" or m < 2):
                        nc.tensor.matmul(ps2[:, :TCH], w2_l[:, m, :], y_t[:, :],
                                         start=True, stop=True)
                        nc.vector.tensor_tensor(h_sb[m][:, gsl], ps2[:, :TCH],
                                                h_sb[m][:, gsl], op=OP.add)
                    else:
                        nc.tensor.matmul(ps2[:, :TCH], w2_l[:, m, :], y_t[:, :],
                                         start=True, stop=False)
                        nc.tensor.matmul(ps2[:, :TCH], ident_r[:], h_sb[m][:, gsl],
                                         start=False, stop=True)
                        nc.scalar.copy(h_sb[m][:, gsl], ps2[:, :TCH])

        # ---- output projection [C, TOK] ----
        for t in range(NT):
            tsl = slice(t * TCH, (t + 1) * TCH)
            ps = pp.tile([128, 512], F32, tag="pp")
            for k in range(KT):
                nc.tensor.matmul(ps[:C, :TCH], wout_sb[:, k, :], h_sb[k][:, tsl],
                                 start=(k == 0), stop=(k == KT - 1))
            o_f = outp.tile([128, TCH], F32, tag="outc")
            if os.environ.get("ANOM_OUTACT", "1") == "1":
                nc.scalar.copy(o_f[:C, :], ps[:C, :TCH])
            else:
                nc.vector.tensor_copy(o_f[:C, :], ps[:C, :TCH])
            nc.sync.dma_start(out=out_d[:, tsl], in_=o_f[:C, :])

    return _legalize_waits(nc)


def _bf(a):
    return np.ascontiguousarray(a).astype(ml_dtypes.bfloat16)


def _r32(a):
    """Round to the reduced-dtype grid (f32r: 10 explicit mantissa bits)."""
    if os.environ.get("ANOM_RDT", "f32r") == "bf16":
        return _bf(a)
    a = np.ascontiguousarray(a, np.float32)
    u = a.view(np.uint32).copy()
    u = (u + 0x1000) & 0xFFFFE000
    return u.view(np.float32)


# o features are written evens-first (heads 0,2,4,6 then 1,3,5,7); Wo's
# input-feature rows are permuted to match.
_PERM_DIN = np.concatenate([np.arange(h * DK, (h + 1) * DK)
                            for h in (0, 2, 4, 6, 1, 3, 5, 7)])


def prep_weights(tok_w, pe, Wq, Wk, Wv, Wo, W1, W2, proj_w):
    """Host-side weight reorganization (shared across cores)."""
    scale = 1.0 / math.sqrt(DK)
    # conv unfold: W_unf[55d + c, o] = tok_w[o, c, d]
    wemb = np.ascontiguousarray(np.transpose(tok_w, (2, 1, 0))).reshape(C3, D)
    # projection weights as lhsT tiles: w[p, l, k, j] = W[l, j, 128k + p]
    def proj_lhsT(W):  # [NL, D_out, D_in] -> [128, NL, KT, D_out]
        return np.ascontiguousarray(
            np.transpose(W, (2, 0, 1)).reshape(KT, 128, NL, W.shape[1])
            .transpose(1, 2, 0, 3))
    eye = np.eye(128, dtype=np.float32)
    m = {
        "identb": _bf(eye), "identr": _r32(eye),
        "wemb0": _r32(wemb[:128]), "wemb1": _r32(wemb[128:]),
        "pe": np.ascontiguousarray(
            np.ascontiguousarray(pe.T).reshape(KT, 128, L).transpose(1, 0, 2)),
        "wq": _r32(proj_lhsT(Wq * scale)),
        "wk": _r32(proj_lhsT(Wk)),
        "wv": _r32(proj_lhsT(Wv)),
        "wo": _bf(proj_lhsT(Wo[:, :, _PERM_DIN])),
        "w1": _r32(proj_lhsT(W1)),
        # w2[p, l, m, j] = W2[l, 128m + j, p]   (p over DFF=64)
        "w2": _r32(np.transpose(W2, (2, 0, 1)).reshape(DFF, NL, KT, 128)),
        # wout[p, k, j] = proj_w[j, 128k + p]
        "wout": _r32(np.ascontiguousarray(proj_w.T).reshape(KT, 128, C)
                     .transpose(1, 0, 2)),
    }
    return m


def prep_xcat(xs):
    """Per-core input: xs [BL, L, C] -> circular-unfolded feature-major
    [165, BL*L], split into [128, .] + [37, .]."""
    xt = np.ascontiguousarray(np.transpose(xs, (2, 0, 1)))    # [C, BL, L]
    rows = [np.roll(xt, 1 - d, axis=2) for d in range(3)]     # x[t+d-1]
    xcat = np.concatenate(rows, axis=0).reshape(C3, TOK)
    return _r32(xcat[:128]), _r32(xcat[128:])


_NC_CACHE = {}


def get_nc():
    if "nc" not in _NC_CACHE:
        _NC_CACHE["nc"] = build_nc()
    return _NC_CACHE["nc"]


def make_in_maps(inputs):
    x = np.asarray(inputs["x"], np.float32)
    wm = prep_weights(np.asarray(inputs["tok_w"], np.float32),
                      np.asarray(inputs["pe"], np.float32),
                      np.asarray(inputs["Wq"], np.float32),
                      np.asarray(inputs["Wk"], np.float32),
                      np.asarray(inputs["Wv"], np.float32),
                      np.asarray(inputs["Wo"], np.float32),
                      np.asarray(inputs["W1"], np.float32),
                      np.asarray(inputs["W2"], np.float32),
                      np.asarray(inputs["proj_w"], np.float32))
    in_maps = []
    for c in range(NCORES):
        x0, x1 = prep_xcat(x[c * BL:(c + 1) * BL])
        in_maps.append({**wm, "xcat0": x0, "xcat1": x1})
    return in_maps


def assemble_out(results):
    # per-core out [C, TOK] feature-major -> [B, L, C]
    outs = [np.asarray(r["out"], np.float32).reshape(C, BL, L).transpose(1, 2, 0)
            for r in results]
    return np.concatenate(outs, axis=0)


def kernel(**inputs) -> np.ndarray:
    nc = get_nc()
    in_maps = make_in_maps(inputs)
    res = run_bass_kernel_spmd(nc, in_maps, core_ids=list(range(NCORES)))
    return assemble_out(res.results)



# revision 10
# speedup vs baseline: 1.0837x; 1.0837x over previous
"""AnomalyTransformer forward pass on 8 Trainium2 NeuronCores.

Data-parallel over batch: each core processes 32 of the 256 batch elements
through the full 3-layer transformer.

Precision strategy: the residual stream h and all projection weights on
the q/k path run in float32r (TF32-like TensorEngine mode, full throughput
at moving-dim >= 256); q/k chunk tiles, the value path and attention
output run in bf16. Softmax logits reach +-38 in layer 3, so an all-bf16
kernel amplifies rounding to ~1.5e-2 relative error; this mix lands at
~6e-3 against the f32 reference.

Layout strategy: the residual stream h is feature-major ([D, tokens], D
split over 4 partition-tiles of 128). Attention uses the scoresT
orientation (scoresT = khT.T @ qhT -> [l_k, l_q]) so softmax normalization
folds into the attention-value matmul via an appended ones-column on V
(column 64 of each head's 65-wide slot accumulates sum(exp)); the
per-token reciprocal is then a per-partition scalar multiply. One
[100, 512] PE-transpose per batch element brings the attention output back
to feature-major for the Wo projection.

Pipeline strategy (what makes PE stay busy):
  - V-projection PSUM is double-buffered (2 banks) so the per-batch V
    matmuls overlap the DVE copies into SBUF.
  - The per-batch attention loop is software-pipelined on PE:
    scores(b) -> vproj(b+1) -> transpose(b-1) -> attnV(b), hiding the
    ACT exp latency and the DVE normalize latency behind PE work.
  - The last batch's transpose and the Wo projection of chunk g are
    deferred into chunk g+1 (independent token slices), hiding the
    normalize/copy chain behind a full projection block.
  - Copies are balanced across ACT and DVE (GPSIMD has no PSUM port and
    every post-matmul op reads PSUM, so it cannot help).
  - FFN runs W1 one chunk ahead of W2, with the residual add split
    between DVE adds and PE identity-matmul accumulation + ACT copyback;
    the output projection is folded into the last layer's FFN loop.
  - The circular-conv token embedding ships x once ([55, 102] padded
    windows per batch) and applies the k=3 unfold as 3 shifted-view
    matmuls, cutting input DMA 3x.

The sigma/prior branch of the reference is dead code (never feeds the
output) and is skipped. Biases in the reference are all zeros and are
skipped.
"""

import sys
import os
for _p in ("/opt/trn_rl_repo", "/root/.axon_site/_ro/trn_rl_repo"):
    if os.path.isdir(_p) and _p not in sys.path:
        sys.path.insert(0, _p)

import math
import numpy as np
import ml_dtypes

import concourse.bass as bass
import concourse.tile as tile
from concourse import mybir
from concourse.bass_utils import run_bass_kernel_spmd
from contextlib import ExitStack

BF16 = mybir.dt.bfloat16
F32 = mybir.dt.float32
F32R = mybir.dt.float32r
AF = mybir.ActivationFunctionType
OP = mybir.AluOpType

# model dims
B, L, C, D, H, NL, DFF = 256, 100, 55, 512, 8, 3, 64
DK = D // H                      # 64
NCORES = 8
BL = B // NCORES                 # 32 batches per core
TOK = BL * L                     # 3200 tokens per core
TCH = 400                        # token chunk (4 batches)
NT = TOK // TCH                  # 8 chunks
CB = TCH // L                    # 4 batches per chunk
KT = D // 128                    # 4 contraction tiles
LP = L + 2                       # padded window for the k=3 circular conv


_NOSTRUCT = ("InstDrain", "InstNoOp", "InstEventSemaphore", "InstHalt")


def _legalize_waits(nc, maxw=1):
    """This container's walrus caps sync-waits at 1 per instruction; move
    extra waits onto preceding same-engine NOPs (one wait each)."""
    cnt = [0]
    for f in nc.m.functions:
        for blk in f.blocks:
            newlist = []
            changed = False
            for ins in blk.instructions:
                si = getattr(ins, "sync_info", None)
                lim = maxw
                if si is not None and si.on_wait and len(si.on_wait) > lim:
                    waits = list(si.on_wait)
                    extra, keep = waits[:-lim], waits[-lim:]
                    for i in range(0, len(extra), 1):
                        cnt[0] += 1
                        nop = mybir.InstNoOp(
                            name=f"I-ws-{cnt[0]}", ins=[], outs=[], engine=ins.engine
                        )
                        nop.sync_info = mybir.SyncInfo(
                            on_wait=extra[i:i + 1], on_update=[]
                        )
                        newlist.append(nop)
                    ins.sync_info = mybir.SyncInfo(
                        on_wait=keep, on_update=list(si.on_update)
                    )
                    changed = True
                newlist.append(ins)
            if changed:
                blk.instructions = newlist
    return nc


def _offset_ap(ap, extra_offset, dims):
    """AP at ap.offset + extra_offset (elements) with free dims `dims`
    ([[step, count], ...]), keeping ap's partition dim."""
    return bass.AP(tensor=ap.tensor, offset=ap.offset + extra_offset,
                   ap=[list(ap.ap[0])] + [list(d) for d in dims])


def build_nc():
    nc = bass.Bass()

    # ---- DRAM parameters (host-prepped) ----
    # xpad[c, t, b, l] : per-chunk circular-padded windows, [55, NT*CB*102]
    xpad_d = nc.declare_dram_parameter("xpad", [C, NT, CB, LP], F32R, isOutput=False)
    wemb_d = nc.declare_dram_parameter("wemb", [C, 3, D], F32R, isOutput=False)
    pe_d = nc.declare_dram_parameter("pe", [128, KT, L], F32, isOutput=False)
    wq_d = nc.declare_dram_parameter("wq", [128, NL, KT, D], F32R, isOutput=False)
    wk_d = nc.declare_dram_parameter("wk", [128, NL, KT, D], F32R, isOutput=False)
    wv_d = nc.declare_dram_parameter("wv", [128, NL, KT, D], F32R, isOutput=False)
    wo_d = nc.declare_dram_parameter("wo", [128, NL, KT, D], BF16, isOutput=False)
    w1_d = nc.declare_dram_parameter("w1", [128, NL, KT, DFF], F32R, isOutput=False)
    w2_d = nc.declare_dram_parameter("w2", [DFF, NL, KT, 128], F32R, isOutput=False)
    wout_d = nc.declare_dram_parameter("wout", [128, KT, C], F32R, isOutput=False)
    identb_d = nc.declare_dram_parameter("identb", [128, 128], BF16, isOutput=False)
    identr_d = nc.declare_dram_parameter("identr", [128, 128], F32R, isOutput=False)
    out_d = nc.declare_dram_parameter("out", [C, TOK], F32, isOutput=True)

    with tile.TileContext(nc) as tc, ExitStack() as stk:
        tc.race_detector_enabled = False
        singles = stk.enter_context(tc.tile_pool(name="singles", bufs=1))
        wp = stk.enter_context(tc.tile_pool(name="wp", bufs=2))
        xp = stk.enter_context(tc.tile_pool(name="xp", bufs=3))
        qp = stk.enter_context(tc.tile_pool(name="qp", bufs=2))
        kp = stk.enter_context(tc.tile_pool(name="kp", bufs=2))
        vp = stk.enter_context(tc.tile_pool(name="vp", bufs=2))
        expp = stk.enter_context(tc.tile_pool(name="expp", bufs=3))
        op_ = stk.enter_context(tc.tile_pool(name="op", bufs=3))
        rp = stk.enter_context(tc.tile_pool(name="rp", bufs=3))
        otp = stk.enter_context(tc.tile_pool(name="otp", bufs=2))
        yp = stk.enter_context(tc.tile_pool(name="yp", bufs=2))
        outp = stk.enter_context(tc.tile_pool(name="outp", bufs=2))
        # psum pools (8 banks total)
        pp = stk.enter_context(tc.tile_pool(name="pp", bufs=3, space="PSUM"))
        scp = stk.enter_context(tc.tile_pool(name="scp", bufs=2, space="PSUM"))
        vpp = stk.enter_context(tc.tile_pool(name="vpp", bufs=2, space="PSUM"))
        fp = stk.enter_context(tc.tile_pool(name="fp", bufs=1, space="PSUM"))

        # ---- persistent SBUF ----
        wemb_sb = singles.tile([C, 3, D], F32R)
        pe_sb = singles.tile([128, KT, L], F32)
        wout_sb = singles.tile([128, KT, C], F32R)
        ident_b = singles.tile([128, 128], BF16)
        ident_r = singles.tile([128, 128], F32R)
        h_sb = [singles.tile([128, TOK], F32R, name=f"h{k}") for k in range(KT)]
        xc = [xp.tile([C, CB, LP], F32R, name=f"xc{t}", tag="xc") for t in range(NT)]

        # DMA priority order: embedding weights + first chunks of x first,
        # then the layer-0 projection weights (interleaved with remaining x
        # chunks), then late-needed singles.
        nc.sync.dma_start(out=wemb_sb[:], in_=wemb_d[:])
        nc.sync.dma_start(out=xc[0][:], in_=xpad_d[:, 0])
        nc.sync.dma_start(out=pe_sb[:], in_=pe_d[:])
        nc.sync.dma_start(out=xc[1][:], in_=xpad_d[:, 1])
        nc.sync.dma_start(out=ident_b[:], in_=identb_d[:])
        nc.sync.dma_start(out=ident_r[:], in_=identr_d[:])
        for t in range(2, NT):
            nc.sync.dma_start(out=xc[t][:], in_=xpad_d[:, t])

        # ---- token embedding: circular conv as 3 shifted matmuls, + pe ----
        for t in range(NT):
            tsl = slice(t * TCH, (t + 1) * TCH)
            for m in range(KT):
                ps = pp.tile([128, 512], F32, tag="pp")
                for d in range(3):
                    # moving: xc[t][:, b, d : d+100] for the 4 batches
                    mv = _offset_ap(xc[t][:, 0, 0], d, [[LP, CB], [1, L]])
                    nc.tensor.matmul(ps[:, :TCH], wemb_sb[:, d, m * 128:(m + 1) * 128],
                                     mv, start=(d == 0), stop=(d == 2))
                pe_b = _offset_ap(pe_sb[:, m, :], 0, [[0, CB], [1, L]])
                nc.vector.tensor_tensor(
                    h_sb[m][:, tsl].rearrange("p (b x) -> p b x", x=L),
                    ps[:, :TCH].rearrange("p (b x) -> p b x", x=L),
                    pe_b, op=OP.add)

        nc.sync.dma_start(out=wout_sb[:], in_=wout_d[:])

        # ---- transformer layers ----
        n_layer_passes = int(os.environ.get("ANOM_LAYERS", str(NL)))
        for lp_i in range(n_layer_passes):
            l = lp_i % NL
            last_layer = lp_i == n_layer_passes - 1
            wq_l = wp.tile([128, KT, D], F32R, tag="wq")
            wk_l = wp.tile([128, KT, D], F32R, tag="wk")
            wv_l = wp.tile([128, KT, D], F32R, tag="wv")
            wo_l = wp.tile([128, KT, D], BF16, tag="wo")
            w1_l = wp.tile([128, KT, DFF], F32R, tag="w1")
            w2_l = wp.tile([DFF, KT, 128], F32R, tag="w2")
            for dst, src in ((wq_l, wq_d), (wk_l, wk_d), (wv_l, wv_d),
                             (wo_l, wo_d), (w1_l, w1_d), (w2_l, w2_d)):
                nc.sync.dma_start(out=dst[:], in_=src[:, l])

            # ---- attention pass over chunks ----
            # pend = (g, ot_all, transpose_o, 3): the last batch's transpose
            # and the whole Wo projection of chunk g are deferred into chunk
            # g+1 so the normalize/copy chain hides behind QKproj(g+1).
            pend = None

            def do_wo(g, ot_all):
                gsl = slice(g * TCH, (g + 1) * TCH)
                for m in range(KT):
                    msl = slice(m * 128, (m + 1) * 128)
                    ps = pp.tile([128, 512], F32, tag="pp")
                    for k in range(KT):
                        nc.tensor.matmul(ps[:, :TCH], wo_l[:, k, msl],
                                         ot_all[:, k, :],
                                         start=(k == 0), stop=(k == KT - 1))
                    nc.vector.tensor_tensor(h_sb[m][:, gsl], ps[:, :TCH],
                                            h_sb[m][:, gsl], op=OP.add)

            def flush_pend():
                g_p, ot_p, tr_p = pend
                tr_p(CB - 1)
                do_wo(g_p, ot_p)

            for g in range(NT):
                gsl = slice(g * TCH, (g + 1) * TCH)
                # Q/K projections for this chunk, feature-major [D, TCH]
                qc = [qp.tile([128, TCH], BF16, name=f"qc{m}", tag=f"qc{m}")
                      for m in range(KT)]
                kc = [kp.tile([128, TCH], BF16, name=f"kc{m}", tag=f"kc{m}")
                      for m in range(KT)]
                for m in range(KT):
                    msl = slice(m * 128, (m + 1) * 128)
                    ps = pp.tile([128, 512], F32, tag="pp")
                    for k in range(KT):
                        nc.tensor.matmul(ps[:, :TCH], wq_l[:, k, msl],
                                         h_sb[k][:, gsl],
                                         start=(k == 0), stop=(k == KT - 1))
                    nc.scalar.copy(qc[m][:], ps[:, :TCH])
                for m in range(KT):
                    msl = slice(m * 128, (m + 1) * 128)
                    ps = pp.tile([128, 512], F32, tag="pp")
                    for k in range(KT):
                        nc.tensor.matmul(ps[:, :TCH], wk_l[:, k, msl],
                                         h_sb[k][:, gsl],
                                         start=(k == 0), stop=(k == KT - 1))
                    nc.scalar.copy(kc[m][:], ps[:, :TCH])

                # previous chunk's deferred transpose + Wo projection: keeps
                # PE fed while this chunk's attention pipeline fills, and
                # gives the previous chunk's normalize/copy chain time.
                if pend is not None:
                    flush_pend()
                    pend = None

                # V tile, token-major per batch (65-stride heads,
                # col 65h+64 = 1 for the softmax-sum trick)
                v_t = vp.tile([128, CB, 8 * 65], BF16, tag="v")
                nc.vector.memset(
                    v_t[:L, :, :].rearrange(
                        "p b (h x) -> p b h x", x=65)[:, :, :, 64:65], 1.0)

                def vproj_mm(bi):
                    b = g * CB + bi
                    bsl = slice(b * L, (b + 1) * L)
                    ps = vpp.tile([128, 512], F32, tag="vps", name="vps")
                    for k in range(KT):
                        nc.tensor.matmul(ps[:L, :], h_sb[k][:, bsl],
                                         wv_l[:, k, :],
                                         start=(k == 0), stop=(k == KT - 1))
                    return ps

                def vcopy(bi, ps, eng):
                    eng(v_t[:L, bi, :].rearrange(
                            "p (h x) -> p h x", x=65)[:, :, :64],
                        ps[:L, :].rearrange("p (h x) -> p h x", x=64))

                ot_all = otp.tile([128, KT, TCH], BF16, tag="ot")
                o_ts = [None] * CB

                def transpose_o(bi):
                    tp = pp.tile([128, 1024], BF16, tag="pp", name="tp")
                    for m in range(KT):
                        nc.tensor.transpose(tp[:, m * L:(m + 1) * L],
                                            o_ts[bi][:L, m * 128:(m + 1) * 128],
                                            ident_b[:L, :L])
                    nc.vector.tensor_copy(
                        _offset_ap(ot_all[:, :, :], bi * L, [[TCH, KT], [1, L]]),
                        tp[:, :KT * L].rearrange("p (m x) -> p m x", x=L))

                vps0 = vproj_mm(0)
                vcopy(0, vps0, nc.vector.tensor_copy)
                for bi in range(CB):
                    csl = slice(bi * L, (bi + 1) * L)
                    # scoresT for 8 heads: even heads -> scA, odd -> scB
                    # (different PE row groups must write different PSUM banks)
                    scA = scp.tile([128, 512], F32, tag="sc")
                    scB = scp.tile([128, 512], F32, tag="sc")
                    for hh in range(8):
                        kt_i, base = divmod(hh * DK, 128)
                        sc = scA if hh % 2 == 0 else scB
                        col = (hh // 2) * 128
                        nc.tensor.matmul(sc[:L, col:col + L],
                                         kc[kt_i][base:base + DK, csl],
                                         qc[kt_i][base:base + DK, csl],
                                         start=True, stop=True)
                    vps_n = vproj_mm(bi + 1) if bi + 1 < CB else None
                    if bi > 0:
                        transpose_o(bi - 1)
                    exp_t = expp.tile([128, 8 * L], BF16, tag="exp")
                    # exp; head hh lands at exp_t cols hh*L
                    nc.scalar.activation(
                        exp_t[:L, :].rearrange("p (h x) -> p h x", x=2 * L)[:, :, :L],
                        scA[:L, :].rearrange("p (h x) -> p h x", x=128)[:, :, :L],
                        AF.Exp)
                    nc.scalar.activation(
                        _offset_ap(exp_t[:L, :], L, [[2 * L, 4], [1, L]]),
                        scB[:L, :].rearrange("p (h x) -> p h x", x=128)[:, :, :L],
                        AF.Exp)
                    # V copy for the next batch: ACT for odd, DVE for even
                    # (the ACT ones are emitted after exp(bi) so they never
                    # delay the exp the next attnV is waiting on)
                    if vps_n is not None:
                        eng = (nc.scalar.copy if (bi + 1) % 2 else
                               nc.vector.tensor_copy)
                        vcopy(bi + 1, vps_n, eng)
                    # oU = expST.T @ [v | 1]  (token-major, col 64 = sum(exp))
                    ouA = pp.tile([128, 512], F32, tag="pp", name="ouA")
                    ouB = pp.tile([128, 512], F32, tag="pp", name="ouB")
                    for hh in range(8):
                        ou = ouA if hh % 2 == 0 else ouB
                        col = (hh // 2) * 128
                        nc.tensor.matmul(ou[:L, col:col + 65],
                                         exp_t[:L, hh * L:(hh + 1) * L],
                                         v_t[:L, bi, hh * 65:(hh + 1) * 65],
                                         start=True, stop=True)
                    r_t = rp.tile([128, 8], F32, tag="r")
                    o_t = op_.tile([128, D], BF16, tag="o")
                    for i, ou in enumerate((ouA, ouB)):
                        nc.vector.reciprocal(
                            r_t[:L, i * 4:(i + 1) * 4],
                            ou[:L, :].rearrange(
                                "p (h x) -> p h x", x=128)[:, :, 64:65])
                        nc.vector.tensor_tensor(
                            o_t[:L, i * 256:(i + 1) * 256].rearrange(
                                "p (h x) -> p h x", x=64),
                            ou[:L, :].rearrange(
                                "p (h x) -> p h x", x=128)[:, :, :64],
                            r_t[:L, i * 4:(i + 1) * 4].rearrange(
                                "p (h x) -> p h x", x=1).broadcast_to([L, 4, 64]),
                            op=OP.mult)
                    o_ts[bi] = o_t
                pend = (g, ot_all, transpose_o)
            flush_pend()

            # ---- FFN pass (W1 runs one chunk ahead of W2) ----
            ps1s = [None] * NT

            def ffn1(g):
                gsl = slice(g * TCH, (g + 1) * TCH)
                ps1 = fp.tile([128, 512], F32, tag="ffn", name="ps1")
                for k in range(KT):
                    nc.tensor.matmul(ps1[:DFF, :TCH], w1_l[:, k, :],
                                     h_sb[k][:, gsl],
                                     start=(k == 0), stop=(k == KT - 1))
                y_t = yp.tile([DFF, TCH], F32R, tag="y")
                nc.scalar.activation(y_t[:, :], ps1[:DFF, :TCH], AF.Gelu)
                return y_t

            def outproj(g):
                tsl = slice(g * TCH, (g + 1) * TCH)
                ps = pp.tile([128, 512], F32, tag="pp")
                for k in range(KT):
                    nc.tensor.matmul(ps[:C, :TCH], wout_sb[:, k, :],
                                     h_sb[k][:, tsl],
                                     start=(k == 0), stop=(k == KT - 1))
                o_f = outp.tile([128, TCH], F32, tag="outc")
                nc.scalar.copy(o_f[:C, :], ps[:C, :TCH])
                nc.sync.dma_start(out=out_d[:, tsl], in_=o_f[:C, :])

            ys = [None] * NT
            ys[0] = ffn1(0)
            for g in range(NT):
                gsl = slice(g * TCH, (g + 1) * TCH)
                if g + 1 < NT:
                    ys[g + 1] = ffn1(g + 1)
                for m in range(KT):
                    ps2 = pp.tile([128, 512], F32, tag="pp", name="ps2")
                    if m < 2:
                        nc.tensor.matmul(ps2[:, :TCH], w2_l[:, m, :], ys[g][:, :],
                                         start=True, stop=True)
                        nc.vector.tensor_tensor(h_sb[m][:, gsl], ps2[:, :TCH],
                                                h_sb[m][:, gsl], op=OP.add)
                    else:
                        # residual folded into PSUM as an identity matmul;
                        # ACT copies back (balances DVE vs ACT in this phase)
                        nc.tensor.matmul(ps2[:, :TCH], w2_l[:, m, :], ys[g][:, :],
                                         start=True, stop=False)
                        nc.tensor.matmul(ps2[:, :TCH], ident_r[:],
                                         h_sb[m][:, gsl],
                                         start=False, stop=True)
                        nc.scalar.copy(h_sb[m][:, gsl], ps2[:, :TCH])
                if last_layer and g >= 1:
                    outproj(g - 1)
            if last_layer:
                outproj(NT - 1)

    return _legalize_waits(nc)


def _bf(a):
    return np.ascontiguousarray(a).astype(ml_dtypes.bfloat16)


def _r32(a):
    """Round to the reduced-dtype grid (f32r: 10 explicit mantissa bits)."""
    a = np.ascontiguousarray(a, np.float32)
    u = a.view(np.uint32).copy()
    u = (u + 0x1000) & 0xFFFFE000
    return u.view(np.float32)


# o features are written evens-first (heads 0,2,4,6 then 1,3,5,7); Wo's
# input-feature rows are permuted to match.
_PERM_DIN = np.concatenate([np.arange(h * DK, (h + 1) * DK)
                            for h in (0, 2, 4, 6, 1, 3, 5, 7)])


def prep_weights(tok_w, pe, Wq, Wk, Wv, Wo, W1, W2, proj_w):
    """Host-side weight reorganization (shared across cores)."""
    scale = 1.0 / math.sqrt(DK)
    # conv as 3 shifted matmuls: wemb[c, d, o] = tok_w[o, c, d]
    wemb = np.ascontiguousarray(np.transpose(tok_w, (1, 2, 0)))  # [C, 3, D]
    # projection weights as lhsT tiles: w[p, l, k, j] = W[l, j, 128k + p]
    def proj_lhsT(W):  # [NL, D_out, D_in] -> [128, NL, KT, D_out]
        return np.ascontiguousarray(
            np.transpose(W, (2, 0, 1)).reshape(KT, 128, NL, W.shape[1])
            .transpose(1, 2, 0, 3))
    eye = np.eye(128, dtype=np.float32)
    m = {
        "identb": _bf(eye), "identr": _r32(eye),
        "wemb": _r32(wemb),
        "pe": np.ascontiguousarray(
            np.ascontiguousarray(pe.T).reshape(KT, 128, L).transpose(1, 0, 2)),
        "wq": _r32(proj_lhsT(Wq * scale)),
        "wk": _r32(proj_lhsT(Wk)),
        "wv": _r32(proj_lhsT(Wv)),
        "wo": _bf(proj_lhsT(Wo[:, :, _PERM_DIN])),
        "w1": _r32(proj_lhsT(W1)),
        # w2[p, l, m, j] = W2[l, 128m + j, p]   (p over DFF=64)
        "w2": _r32(np.transpose(W2, (2, 0, 1)).reshape(DFF, NL, KT, 128)),
        # wout[p, k, j] = proj_w[j, 128k + p]
        "wout": _r32(np.ascontiguousarray(proj_w.T).reshape(KT, 128, C)
                     .transpose(1, 0, 2)),
    }
    return m


def prep_xpad(xs):
    """Per-core input: xs [BL, L, C] -> feature-major circular-padded
    windows [C, NT, CB, L+2]."""
    xt = np.transpose(xs, (2, 0, 1))                     # [C, BL, L]
    xpad = np.empty((C, BL, LP), np.float32)
    xpad[:, :, 1:L + 1] = xt
    xpad[:, :, 0] = xt[:, :, L - 1]
    xpad[:, :, L + 1] = xt[:, :, 0]
    return _r32(xpad.reshape(C, NT, CB, LP))


_NC_CACHE = {}


def get_nc():
    if "nc" not in _NC_CACHE:
        _NC_CACHE["nc"] = build_nc()
    return _NC_CACHE["nc"]


def make_in_maps(inputs):
    x = np.asarray(inputs["x"], np.float32)
    wm = prep_weights(np.asarray(inputs["tok_w"], np.float32),
                      np.asarray(inputs["pe"], np.float32),
                      np.asarray(inputs["Wq"], np.float32),
                      np.asarray(inputs["Wk"], np.float32),
                      np.asarray(inputs["Wv"], np.float32),
                      np.asarray(inputs["Wo"], np.float32),
                      np.asarray(inputs["W1"], np.float32),
                      np.asarray(inputs["W2"], np.float32),
                      np.asarray(inputs["proj_w"], np.float32))
    in_maps = []
    for c in range(NCORES):
        in_maps.append({**wm, "xpad": prep_xpad(x[c * BL:(c + 1) * BL])})
    return in_maps


def assemble_out(results):
    # per-core out [C, TOK] feature-major -> [B, L, C]
    outs = [np.asarray(r["out"], np.float32).reshape(C, BL, L).transpose(1, 2, 0)
            for r in results]
    return np.concatenate(outs, axis=0)


def kernel(**inputs) -> np.ndarray:
    nc = get_nc()
    in_maps = make_in_maps(inputs)
    res = run_bass_kernel_spmd(nc, in_maps, core_ids=list(range(NCORES)))
    return assemble_out(res.results)


# revision 13
# speedup vs baseline: 1.1517x; 1.0627x over previous
"""AnomalyTransformer forward pass on 8 Trainium2 NeuronCores.

Data-parallel over batch: each core processes 32 of the 256 batch elements
through the full 3-layer transformer.

Precision strategy: the residual stream h and all projection weights on
the q/k path run in float32r (TF32-like TensorEngine mode, full throughput
at moving-dim >= 256); q/k chunk tiles, the value path and attention
output run in bf16. Softmax logits reach +-38 in layer 3, so an all-bf16
kernel amplifies rounding to ~1.5e-2 relative error; this mix lands at
~6e-3 against the f32 reference.

Layout strategy: the residual stream h is feature-major ([D, tokens], D
split over 4 partition-tiles of 128). Attention uses the scoresT
orientation (scoresT = khT.T @ qhT -> [l_k, l_q]) so softmax normalization
folds into the attention-value matmul via an appended ones-column on V
(column 64 of each head's 65-wide slot accumulates sum(exp)); the
per-token reciprocal is then a per-partition scalar multiply. One
[100, 512] PE-transpose per batch element brings the attention output back
to feature-major for the Wo projection.

Pipeline strategy (what makes PE stay busy):
  - V-projection PSUM is double-buffered (2 banks) so the per-batch V
    matmuls overlap the DVE copies into SBUF.
  - The per-batch attention loop is software-pipelined on PE:
    scores(b) -> vproj(b+1) -> transpose(b-1) -> attnV(b), hiding the
    ACT exp latency and the DVE normalize latency behind PE work.
  - The last batch's transpose and the Wo projection of chunk g are
    deferred into chunk g+1 (independent token slices), hiding the
    normalize/copy chain behind a full projection block.
  - Copies are balanced across ACT and DVE (GPSIMD has no PSUM port and
    every post-matmul op reads PSUM, so it cannot help).
  - FFN runs W1 one chunk ahead of W2, with the residual add split
    between DVE adds and PE identity-matmul accumulation + ACT copyback;
    the output projection is folded into the last layer's FFN loop.
  - The circular-conv token embedding ships x once ([55, 102] padded
    windows per batch) and applies the k=3 unfold as 3 shifted-view
    matmuls, cutting input DMA 3x.

The sigma/prior branch of the reference is dead code (never feeds the
output) and is skipped. Biases in the reference are all zeros and are
skipped.
"""

import sys
import os
for _p in ("/opt/trn_rl_repo", "/root/.axon_site/_ro/trn_rl_repo"):
    if os.path.isdir(_p) and _p not in sys.path:
        sys.path.insert(0, _p)

import math
import numpy as np
import ml_dtypes

import concourse.bass as bass
import concourse.tile as tile
from concourse import mybir
from concourse.bass_utils import run_bass_kernel_spmd
from contextlib import ExitStack

BF16 = mybir.dt.bfloat16
F32 = mybir.dt.float32
F32R = mybir.dt.float32r
AF = mybir.ActivationFunctionType
OP = mybir.AluOpType

# model dims
B, L, C, D, H, NL, DFF = 256, 100, 55, 512, 8, 3, 64
DK = D // H                      # 64
NCORES = 8
BL = B // NCORES                 # 32 batches per core
TOK = BL * L                     # 3200 tokens per core
TCH = 400                        # token chunk (4 batches)
NT = TOK // TCH                  # 8 chunks
CB = TCH // L                    # 4 batches per chunk
KT = D // 128                    # 4 contraction tiles
LP = L + 2                       # padded window for the k=3 circular conv


_NOSTRUCT = ("InstDrain", "InstNoOp", "InstEventSemaphore", "InstHalt")


def _legalize_waits(nc, maxw=1):
    """This container's walrus caps sync-waits at 1 per instruction; move
    extra waits onto preceding same-engine NOPs (one wait each)."""
    cnt = [0]
    for f in nc.m.functions:
        for blk in f.blocks:
            newlist = []
            changed = False
            for ins in blk.instructions:
                si = getattr(ins, "sync_info", None)
                lim = maxw
                if si is not None and si.on_wait and len(si.on_wait) > lim:
                    waits = list(si.on_wait)
                    extra, keep = waits[:-lim], waits[-lim:]
                    for i in range(0, len(extra), 1):
                        cnt[0] += 1
                        nop = mybir.InstNoOp(
                            name=f"I-ws-{cnt[0]}", ins=[], outs=[], engine=ins.engine
                        )
                        nop.sync_info = mybir.SyncInfo(
                            on_wait=extra[i:i + 1], on_update=[]
                        )
                        newlist.append(nop)
                    ins.sync_info = mybir.SyncInfo(
                        on_wait=keep, on_update=list(si.on_update)
                    )
                    changed = True
                newlist.append(ins)
            if changed:
                blk.instructions = newlist
    return nc


def _offset_ap(ap, extra_offset, dims):
    """AP at ap.offset + extra_offset (elements) with free dims `dims`
    ([[step, count], ...]), keeping ap's partition dim."""
    return bass.AP(tensor=ap.tensor, offset=ap.offset + extra_offset,
                   ap=[list(ap.ap[0])] + [list(d) for d in dims])


def build_nc():
    nc = bass.Bass()

    # ---- DRAM parameters (host-prepped) ----
    # xpad[c, t, b, l] : per-chunk circular-padded windows, [55, NT*CB*102]
    xpad_d = nc.declare_dram_parameter("xpad", [C, NT, CB, LP], F32R, isOutput=False)
    wemb_d = nc.declare_dram_parameter("wemb", [C, 3, D], F32R, isOutput=False)
    pe_d = nc.declare_dram_parameter("pe", [128, KT, L], F32, isOutput=False)
    wq_d = nc.declare_dram_parameter("wq", [128, NL, KT, D], F32R, isOutput=False)
    wk_d = nc.declare_dram_parameter("wk", [128, NL, KT, D], F32R, isOutput=False)
    wv_d = nc.declare_dram_parameter("wv", [128, NL, KT, D], F32R, isOutput=False)
    wo_d = nc.declare_dram_parameter("wo", [128, NL, KT, D], BF16, isOutput=False)
    w1_d = nc.declare_dram_parameter("w1", [128, NL, KT, DFF], F32R, isOutput=False)
    w2_d = nc.declare_dram_parameter("w2", [DFF, NL, KT, 128], F32R, isOutput=False)
    wout_d = nc.declare_dram_parameter("wout", [128, KT, C], F32R, isOutput=False)
    identb_d = nc.declare_dram_parameter("identb", [128, 128], BF16, isOutput=False)
    identr_d = nc.declare_dram_parameter("identr", [128, 128], F32R, isOutput=False)
    out_d = nc.declare_dram_parameter("out", [C, TOK], F32, isOutput=True)

    with tile.TileContext(nc) as tc, ExitStack() as stk:
        tc.race_detector_enabled = False
        singles = stk.enter_context(tc.tile_pool(name="singles", bufs=1))
        wp = stk.enter_context(tc.tile_pool(name="wp", bufs=2))
        xp = stk.enter_context(tc.tile_pool(name="xp", bufs=3))
        qp = stk.enter_context(tc.tile_pool(name="qp", bufs=2))
        kp = stk.enter_context(tc.tile_pool(name="kp", bufs=2))
        vp = stk.enter_context(tc.tile_pool(name="vp", bufs=2))
        expp = stk.enter_context(tc.tile_pool(name="expp", bufs=3))
        op_ = stk.enter_context(tc.tile_pool(name="op", bufs=3))
        rp = stk.enter_context(tc.tile_pool(name="rp", bufs=3))
        otp = stk.enter_context(tc.tile_pool(name="otp", bufs=2))
        yp = stk.enter_context(tc.tile_pool(name="yp", bufs=2))
        outp = stk.enter_context(tc.tile_pool(name="outp", bufs=2))
        # psum pools (8 banks total); the FFN W1 accumulator borrows the
        # attention-idle vpp pool
        pp = stk.enter_context(tc.tile_pool(name="pp", bufs=3, space="PSUM"))
        scp = stk.enter_context(tc.tile_pool(name="scp", bufs=3, space="PSUM"))
        vpp = stk.enter_context(tc.tile_pool(name="vpp", bufs=2, space="PSUM"))

        # ---- persistent SBUF ----
        wemb_sb = singles.tile([C, 3, D], F32R)
        pe_sb = singles.tile([128, KT, L], F32)
        wout_sb = singles.tile([128, KT, C], F32R)
        ident_b = singles.tile([128, 128], BF16)
        ident_r = singles.tile([128, 128], F32R)
        h_sb = [singles.tile([128, TOK], F32R, name=f"h{k}") for k in range(KT)]
        xc = [xp.tile([C, CB, LP], F32R, name=f"xc{t}", tag="xc") for t in range(NT)]

        # DMA priority order: embedding weights + first chunks of x first,
        # then the layer-0 projection weights (interleaved with remaining x
        # chunks), then late-needed singles.
        nc.sync.dma_start(out=wemb_sb[:], in_=wemb_d[:])
        nc.sync.dma_start(out=xc[0][:], in_=xpad_d[:, 0])
        nc.sync.dma_start(out=pe_sb[:], in_=pe_d[:])
        nc.sync.dma_start(out=xc[1][:], in_=xpad_d[:, 1])
        nc.sync.dma_start(out=ident_b[:], in_=identb_d[:])
        nc.sync.dma_start(out=ident_r[:], in_=identr_d[:])
        for t in range(2, NT):
            nc.sync.dma_start(out=xc[t][:], in_=xpad_d[:, t])

        # ---- token embedding: circular conv as 3 shifted matmuls, + pe ----
        for t in range(NT):
            tsl = slice(t * TCH, (t + 1) * TCH)
            for m in range(KT):
                ps = pp.tile([128, 512], F32, tag="pp")
                for d in range(3):
                    # moving: xc[t][:, b, d : d+100] for the 4 batches
                    mv = _offset_ap(xc[t][:, 0, 0], d, [[LP, CB], [1, L]])
                    nc.tensor.matmul(ps[:, :TCH], wemb_sb[:, d, m * 128:(m + 1) * 128],
                                     mv, start=(d == 0), stop=(d == 2))
                pe_b = _offset_ap(pe_sb[:, m, :], 0, [[0, CB], [1, L]])
                nc.vector.tensor_tensor(
                    h_sb[m][:, tsl].rearrange("p (b x) -> p b x", x=L),
                    ps[:, :TCH].rearrange("p (b x) -> p b x", x=L),
                    pe_b, op=OP.add)

        nc.sync.dma_start(out=wout_sb[:], in_=wout_d[:])

        # ---- transformer layers ----
        n_layer_passes = int(os.environ.get("ANOM_LAYERS", str(NL)))
        for lp_i in range(n_layer_passes):
            l = lp_i % NL
            last_layer = lp_i == n_layer_passes - 1
            wq_l = wp.tile([128, KT, D], F32R, tag="wq")
            wk_l = wp.tile([128, KT, D], F32R, tag="wk")
            wv_l = wp.tile([128, KT, D], F32R, tag="wv")
            wo_l = wp.tile([128, KT, D], BF16, tag="wo")
            w1_l = wp.tile([128, KT, DFF], F32R, tag="w1")
            w2_l = wp.tile([DFF, KT, 128], F32R, tag="w2")
            for dst, src in ((wq_l, wq_d), (wk_l, wk_d), (wv_l, wv_d),
                             (wo_l, wo_d), (w1_l, w1_d), (w2_l, w2_d)):
                nc.sync.dma_start(out=dst[:], in_=src[:, l])

            # ---- attention pass over chunks ----
            # pend = (g, ot_all, transpose_o, 3): the last batch's transpose
            # and the whole Wo projection of chunk g are deferred into chunk
            # g+1 so the normalize/copy chain hides behind QKproj(g+1).
            pend = None

            def do_wo(g, ot_all):
                gsl = slice(g * TCH, (g + 1) * TCH)
                for m in range(KT):
                    msl = slice(m * 128, (m + 1) * 128)
                    ps = pp.tile([128, 512], F32, tag="pp")
                    for k in range(KT):
                        nc.tensor.matmul(ps[:, :TCH], wo_l[:, k, msl],
                                         ot_all[:, k, :],
                                         start=(k == 0), stop=(k == KT - 1))
                    nc.vector.tensor_tensor(h_sb[m][:, gsl], ps[:, :TCH],
                                            h_sb[m][:, gsl], op=OP.add)

            def flush_pend():
                g_p, ot_p, tr_p = pend
                tr_p(CB - 1)
                do_wo(g_p, ot_p)

            for g in range(NT):
                gsl = slice(g * TCH, (g + 1) * TCH)

                # V tile, token-major per batch (65-stride heads,
                # col 65h+64 = 1 for the softmax-sum trick)
                v_t = vp.tile([128, CB, 8 * 65], BF16, tag="v")
                nc.vector.memset(
                    v_t[:L, :, :].rearrange(
                        "p b (h x) -> p b h x", x=65)[:, :, :, 64:65], 1.0)

                def vproj_mm(bi):
                    b = g * CB + bi
                    bsl = slice(b * L, (b + 1) * L)
                    ps = vpp.tile([128, 512], F32, tag="vps", name="vps")
                    for k in range(KT):
                        nc.tensor.matmul(ps[:L, :], h_sb[k][:, bsl],
                                         wv_l[:, k, :],
                                         start=(k == 0), stop=(k == KT - 1))
                    return ps

                def vcopy(bi, ps, eng):
                    eng(v_t[:L, bi, :].rearrange(
                            "p (h x) -> p h x", x=65)[:, :, :64],
                        ps[:L, :].rearrange("p (h x) -> p h x", x=64))

                # batch-0 V projection first: PE filler that covers the
                # chunk-boundary wait on the previous batch's normalize
                # (the first QK psum slot is freed by it)
                vps0 = vproj_mm(0)
                vcopy(0, vps0, nc.vector.tensor_copy)

                # Q/K projections for this chunk, feature-major [D, TCH]
                qc = [qp.tile([128, TCH], BF16, name=f"qc{m}", tag=f"qc{m}")
                      for m in range(KT)]
                kc = [kp.tile([128, TCH], BF16, name=f"kc{m}", tag=f"kc{m}")
                      for m in range(KT)]
                for m in range(KT):
                    msl = slice(m * 128, (m + 1) * 128)
                    ps = pp.tile([128, 512], F32, tag="pp")
                    for k in range(KT):
                        nc.tensor.matmul(ps[:, :TCH], wq_l[:, k, msl],
                                         h_sb[k][:, gsl],
                                         start=(k == 0), stop=(k == KT - 1))
                    nc.scalar.copy(qc[m][:], ps[:, :TCH])
                for m in range(KT):
                    msl = slice(m * 128, (m + 1) * 128)
                    ps = pp.tile([128, 512], F32, tag="pp")
                    for k in range(KT):
                        nc.tensor.matmul(ps[:, :TCH], wk_l[:, k, msl],
                                         h_sb[k][:, gsl],
                                         start=(k == 0), stop=(k == KT - 1))
                    nc.scalar.copy(kc[m][:], ps[:, :TCH])

                # previous chunk's deferred transpose + Wo projection: keeps
                # PE fed while this chunk's attention pipeline fills, and
                # gives the previous chunk's normalize/copy chain time.
                if pend is not None:
                    flush_pend()
                    pend = None

                ot_all = otp.tile([128, KT, TCH], BF16, tag="ot")
                o_ts = [None] * CB

                def transpose_o(bi):
                    tp = pp.tile([128, 1024], BF16, tag="pp", name="tp")
                    for m in range(KT):
                        nc.tensor.transpose(tp[:, m * L:(m + 1) * L],
                                            o_ts[bi][:L, m * 128:(m + 1) * 128],
                                            ident_b[:L, :L])
                    nc.vector.tensor_copy(
                        _offset_ap(ot_all[:, :, :], bi * L, [[TCH, KT], [1, L]]),
                        tp[:, :KT * L].rearrange("p (m x) -> p m x", x=L))

                for bi in range(CB):
                    csl = slice(bi * L, (bi + 1) * L)
                    # scoresT for 8 heads: even heads -> scA, odd -> scB
                    # (different PE row groups must write different PSUM banks)
                    scA = scp.tile([128, 512], F32, tag="sc")
                    scB = scp.tile([128, 512], F32, tag="sc")
                    for hh in range(8):
                        kt_i, base = divmod(hh * DK, 128)
                        sc = scA if hh % 2 == 0 else scB
                        col = (hh // 2) * 128
                        nc.tensor.matmul(sc[:L, col:col + L],
                                         kc[kt_i][base:base + DK, csl],
                                         qc[kt_i][base:base + DK, csl],
                                         start=True, stop=True)
                    vps_n = vproj_mm(bi + 1) if bi + 1 < CB else None
                    exp_t = expp.tile([128, 8 * L], BF16, tag="exp")
                    # exp; head hh lands at exp_t cols hh*L
                    nc.scalar.activation(
                        exp_t[:L, :].rearrange("p (h x) -> p h x", x=2 * L)[:, :, :L],
                        scA[:L, :].rearrange("p (h x) -> p h x", x=128)[:, :, :L],
                        AF.Exp)
                    nc.scalar.activation(
                        _offset_ap(exp_t[:L, :], L, [[2 * L, 4], [1, L]]),
                        scB[:L, :].rearrange("p (h x) -> p h x", x=128)[:, :, :L],
                        AF.Exp)
                    # V copy for the next batch: ACT for odd, DVE for even
                    # (the ACT ones are emitted after exp(bi) so they never
                    # delay the exp the next attnV is waiting on)
                    if vps_n is not None:
                        eng = (nc.scalar.copy if (bi + 1) % 2 else
                               nc.vector.tensor_copy)
                        vcopy(bi + 1, vps_n, eng)
                    # oU = expST.T @ [v | 1]  (token-major, col 64 = sum(exp))
                    ouA = pp.tile([128, 512], F32, tag="pp", name="ouA")
                    ouB = pp.tile([128, 512], F32, tag="pp", name="ouB")
                    for hh in range(8):
                        ou = ouA if hh % 2 == 0 else ouB
                        col = (hh // 2) * 128
                        nc.tensor.matmul(ou[:L, col:col + 65],
                                         exp_t[:L, hh * L:(hh + 1) * L],
                                         v_t[:L, bi, hh * 65:(hh + 1) * 65],
                                         start=True, stop=True)
                    r_t = rp.tile([128, 8], F32, tag="r")
                    o_t = op_.tile([128, D], BF16, tag="o")
                    for i, ou in enumerate((ouA, ouB)):
                        nc.vector.reciprocal(
                            r_t[:L, i * 4:(i + 1) * 4],
                            ou[:L, :].rearrange(
                                "p (h x) -> p h x", x=128)[:, :, 64:65])
                        nc.vector.tensor_tensor(
                            o_t[:L, i * 256:(i + 1) * 256].rearrange(
                                "p (h x) -> p h x", x=64),
                            ou[:L, :].rearrange(
                                "p (h x) -> p h x", x=128)[:, :, :64],
                            r_t[:L, i * 4:(i + 1) * 4].rearrange(
                                "p (h x) -> p h x", x=1).broadcast_to([L, 4, 64]),
                            op=OP.mult)
                    o_ts[bi] = o_t
                    # previous batch's transpose last: by now its normalize
                    # has had a full iteration of PE work to complete
                    if bi > 0:
                        transpose_o(bi - 1)
                pend = (g, ot_all, transpose_o)
            flush_pend()

            # ---- FFN pass (W1 runs one chunk ahead of W2) ----
            ps1s = [None] * NT

            def ffn1(g):
                gsl = slice(g * TCH, (g + 1) * TCH)
                ps1 = vpp.tile([128, 512], F32, tag="vps", name="ps1")
                for k in range(KT):
                    nc.tensor.matmul(ps1[:DFF, :TCH], w1_l[:, k, :],
                                     h_sb[k][:, gsl],
                                     start=(k == 0), stop=(k == KT - 1))
                y_t = yp.tile([DFF, TCH], F32R, tag="y")
                nc.scalar.activation(y_t[:, :], ps1[:DFF, :TCH], AF.Gelu)
                return y_t

            def outproj(g):
                tsl = slice(g * TCH, (g + 1) * TCH)
                ps = pp.tile([128, 512], F32, tag="pp")
                for k in range(KT):
                    nc.tensor.matmul(ps[:C, :TCH], wout_sb[:, k, :],
                                     h_sb[k][:, tsl],
                                     start=(k == 0), stop=(k == KT - 1))
                o_f = outp.tile([128, TCH], F32, tag="outc")
                nc.scalar.copy(o_f[:C, :], ps[:C, :TCH])
                nc.sync.dma_start(out=out_d[:, tsl], in_=o_f[:C, :])

            ys = [None] * NT
            ys[0] = ffn1(0)
            for g in range(NT):
                gsl = slice(g * TCH, (g + 1) * TCH)
                if g + 1 < NT:
                    ys[g + 1] = ffn1(g + 1)
                for m in range(KT):
                    ps2 = pp.tile([128, 512], F32, tag="pp", name="ps2")
                    if m < 2:
                        nc.tensor.matmul(ps2[:, :TCH], w2_l[:, m, :], ys[g][:, :],
                                         start=True, stop=True)
                        nc.vector.tensor_tensor(h_sb[m][:, gsl], ps2[:, :TCH],
                                                h_sb[m][:, gsl], op=OP.add)
                    else:
                        # residual folded into PSUM as an identity matmul;
                        # ACT copies back (balances DVE vs ACT in this phase)
                        nc.tensor.matmul(ps2[:, :TCH], w2_l[:, m, :], ys[g][:, :],
                                         start=True, stop=False)
                        nc.tensor.matmul(ps2[:, :TCH], ident_r[:],
                                         h_sb[m][:, gsl],
                                         start=False, stop=True)
                        nc.scalar.copy(h_sb[m][:, gsl], ps2[:, :TCH])
                if last_layer and g >= 1:
                    outproj(g - 1)
            if last_layer:
                outproj(NT - 1)

    return _legalize_waits(nc)


def _bf(a):
    return np.ascontiguousarray(a).astype(ml_dtypes.bfloat16)


def _r32(a):
    """Round to the reduced-dtype grid (f32r: 10 explicit mantissa bits)."""
    a = np.ascontiguousarray(a, np.float32)
    u = a.view(np.uint32).copy()
    u = (u + 0x1000) & 0xFFFFE000
    return u.view(np.float32)


# o features are written evens-first (heads 0,2,4,6 then 1,3,5,7); Wo's
# input-feature rows are permuted to match.
_PERM_DIN = np.concatenate([np.arange(h * DK, (h + 1) * DK)
                            for h in (0, 2, 4, 6, 1, 3, 5, 7)])


def prep_weights(tok_w, pe, Wq, Wk, Wv, Wo, W1, W2, proj_w):
    """Host-side weight reorganization (shared across cores)."""
    scale = 1.0 / math.sqrt(DK)
    # conv as 3 shifted matmuls: wemb[c, d, o] = tok_w[o, c, d]
    wemb = np.ascontiguousarray(np.transpose(tok_w, (1, 2, 0)))  # [C, 3, D]
    # projection weights as lhsT tiles: w[p, l, k, j] = W[l, j, 128k + p]
    def proj_lhsT(W):  # [NL, D_out, D_in] -> [128, NL, KT, D_out]
        return np.ascontiguousarray(
            np.transpose(W, (2, 0, 1)).reshape(KT, 128, NL, W.shape[1])
            .transpose(1, 2, 0, 3))
    eye = np.eye(128, dtype=np.float32)
    m = {
        "identb": _bf(eye), "identr": _r32(eye),
        "wemb": _r32(wemb),
        "pe": np.ascontiguousarray(
            np.ascontiguousarray(pe.T).reshape(KT, 128, L).transpose(1, 0, 2)),
        "wq": _r32(proj_lhsT(Wq * scale)),
        "wk": _r32(proj_lhsT(Wk)),
        "wv": _r32(proj_lhsT(Wv)),
        "wo": _bf(proj_lhsT(Wo[:, :, _PERM_DIN])),
        "w1": _r32(proj_lhsT(W1)),
        # w2[p, l, m, j] = W2[l, 128m + j, p]   (p over DFF=64)
        "w2": _r32(np.transpose(W2, (2, 0, 1)).reshape(DFF, NL, KT, 128)),
        # wout[p, k, j] = proj_w[j, 128k + p]
        "wout": _r32(np.ascontiguousarray(proj_w.T).reshape(KT, 128, C)
                     .transpose(1, 0, 2)),
    }
    return m


def prep_xpad(xs):
    """Per-core input: xs [BL, L, C] -> feature-major circular-padded
    windows [C, NT, CB, L+2]."""
    xt = np.transpose(xs, (2, 0, 1))                     # [C, BL, L]
    xpad = np.empty((C, BL, LP), np.float32)
    xpad[:, :, 1:L + 1] = xt
    xpad[:, :, 0] = xt[:, :, L - 1]
    xpad[:, :, L + 1] = xt[:, :, 0]
    return _r32(xpad.reshape(C, NT, CB, LP))


_NC_CACHE = {}


def get_nc():
    if "nc" not in _NC_CACHE:
        _NC_CACHE["nc"] = build_nc()
    return _NC_CACHE["nc"]


def make_in_maps(inputs):
    x = np.asarray(inputs["x"], np.float32)
    wm = prep_weights(np.asarray(inputs["tok_w"], np.float32),
                      np.asarray(inputs["pe"], np.float32),
                      np.asarray(inputs["Wq"], np.float32),
                      np.asarray(inputs["Wk"], np.float32),
                      np.asarray(inputs["Wv"], np.float32),
                      np.asarray(inputs["Wo"], np.float32),
                      np.asarray(inputs["W1"], np.float32),
                      np.asarray(inputs["W2"], np.float32),
                      np.asarray(inputs["proj_w"], np.float32))
    in_maps = []
    for c in range(NCORES):
        in_maps.append({**wm, "xpad": prep_xpad(x[c * BL:(c + 1) * BL])})
    return in_maps


def assemble_out(results):
    # per-core out [C, TOK] feature-major -> [B, L, C]
    outs = [np.asarray(r["out"], np.float32).reshape(C, BL, L).transpose(1, 2, 0)
            for r in results]
    return np.concatenate(outs, axis=0)


def kernel(**inputs) -> np.ndarray:
    nc = get_nc()
    in_maps = make_in_maps(inputs)
    res = run_bass_kernel_spmd(nc, in_maps, core_ids=list(range(NCORES)))
    return assemble_out(res.results)
